# revision 59
# baseline (speedup 1.0000x reference)
"""Mistral GQA self-attention on 8 NeuronCores, tensor-parallel over heads.

Sharding: core c owns q-heads [4c, 4c+4) and kv-head c (q_group-aligned).
Each core computes its heads' attention output and a partial output
projection (rows 512c..512c+512 of wo); host sums the 8 partials (fp16).

Device scheme (causal fast path, v2):
  - all weights (wq|wk|wv as w_sb, wo as wo_sb) resident in SBUF across
    both batches; cos/sin fp16 with sin sign folded host-side.
  - projections in "S^T" layout: prj[d, t] = w_chunk.T @ xT_chunk, PSUM
    waves of 2 outputs x 32 ic-chunks; RoPE fully on DVE in fp16 (4x
    mode) with the partition-rotated copy via 2 SBUF->SBUF DMAs.
  - V transposed to [t, d] via XBAR dma_start_transpose (no PE/PSUM).
  - scores transposed: S^T[k, q] = K^T_blk.T @ Q^T (PSUM); causal
    strictly-upper k-blocks skipped; the triangular 128x128 diagonal
    sub-block is masked by MULTIPLYING exp output with a 0/1 triangle
    (exactly matches exp(-30000)=0 of the reference affine mask).
  - softmax without max-subtraction: exp(s-4) on ACT; Z via DVE fp16
    accumulation of e-blocks + ONE ones-matmul per (h, qg) instead of
    one per k-block (16x fewer Z matmuls on PE).
  - PV: O^T[d, q] = sum_k V[k,d].T @ expS^T, lagged 3 blocks behind the
    score stream so exp latency never stalls the PE.
  - WO: out[t, c] accumulated over the 4 head-chunks; fp16 output DMA;
    PSUM->SBUF staging copies on DVE (keeps ACT exp-only).
  - emission is software-pipelined: proj/WO matmul quanta are woven into
    the ACT-bound attention stream via a debt counter so the PE never
    starves; all DMAs on the SP queue (ACT-queue DMAs stall exp dispatch
    while acquiring HWDGE); weights/x DMAs chunk-interleaved at startup.
Matmul operands fp16 (1 cyc/row), fp32 PSUM accumulation.
"""
import sys

sys.path.insert(0, "/opt/trn_rl_repo")
import numpy as np

B, T, H, D = 2, 2048, 32, 128
Q_GROUP = 4
H_KV = H // Q_GROUP
INNER = H * D          # 4096
NCORES = 8
HPC = H // NCORES      # 4 q-heads per core
ATTN_SCALE = 1.0 / np.sqrt(D)
BT = B * T             # 4096
QG = 512               # q-group (free dim of attention matmuls)
NQG = T // QG          # 4
NKB = T // 128         # 16 k-blocks
NIC = INNER // 128     # 32 contraction chunks
NTB = T // 128         # 16 token blocks per batch
NCG = INNER // 512     # 8 output column groups

_built = {}


def _split_waits(nc, mybir):
    """Walrus codegen in this container supports only 1 sync-wait per ISA
    instruction; hoist extra waits onto preceding same-engine EventSemaphore
    instructions (1 wait each)."""
    for f in nc.m.functions:
        for bb in f.blocks:
            new = []
            for inst in bb.instructions:
                si = inst.sync_info
                ow = list(si.on_wait) if si is not None and si.on_wait else []
                if len(ow) > 1:
                    for wi, w in enumerate(ow):
                        ev = mybir.InstEventSemaphore(
                            name=f"{inst.name}-wsplit{wi}",
                            ins=[], outs=[],
                            sync_info=mybir.SyncInfo(on_wait=[w], on_update=[]),
                        )
                        ev.engine = inst.engine
                        ev.debug = inst.debug
                        new.append(ev)
                    inst.sync_info = mybir.SyncInfo(
                        on_wait=[], on_update=list(si.on_update or []))
                new.append(inst)
            bb.instructions[:] = new


def _build_causal():
    import concourse.bass as bass
    import concourse.bass_isa as bass_isa
    import concourse.mybir as mybir
    import concourse.tile as tile
    from collections import deque
    from contextlib import ExitStack

    F32 = mybir.dt.float32
    FR = mybir.dt.float16
    EXP = mybir.ActivationFunctionType.Exp

    nc = bass.Bass(trn_type="TRN2", target_bir_lowering=False, debug=False)
    xT = nc.dram_tensor("xT", [INNER, BT], FR, kind="ExternalInput").ap()
    wq = nc.dram_tensor("wq", [INNER, HPC * D], FR, kind="ExternalInput").ap()
    wkv = nc.dram_tensor("wkv", [INNER, 2 * D], FR, kind="ExternalInput").ap()
    wo = nc.dram_tensor("wo", [HPC * D, INNER], FR, kind="ExternalInput").ap()
    cosT = nc.dram_tensor("cosT", [D, T], FR, kind="ExternalInput").ap()
    sinTs = nc.dram_tensor("sinTs", [D, T], FR, kind="ExternalInput").ap()
    triT = nc.dram_tensor("triT", [128, 128], FR, kind="ExternalInput").ap()
    out = nc.dram_tensor("out", [BT, INNER], FR, kind="ExternalOutput").ap()

    with tile.TileContext(nc) as tc, ExitStack() as ctx:
        const = ctx.enter_context(tc.tile_pool(name="const", bufs=1))
        data = ctx.enter_context(tc.tile_pool(name="data", bufs=1))
        xp = ctx.enter_context(tc.tile_pool(name="xp", bufs=10))
        wk_ = ctx.enter_context(tc.tile_pool(name="wk", bufs=2))
        pps = ctx.enter_context(tc.tile_pool(name="pps", bufs=1, space="PSUM"))
        sps = ctx.enter_context(tc.tile_pool(name="sps", bufs=2, space="PSUM"))
        ops = ctx.enter_context(tc.tile_pool(name="ops", bufs=2, space="PSUM"))
        wps = ctx.enter_context(tc.tile_pool(name="wps", bufs=2, space="PSUM"))

        # ---- batch-invariant constants ----
        biasm4 = const.tile([128, 1], F32)
        nc.gpsimd.memset(biasm4, -4.0)
        w_sb = const.tile([128, NIC, (HPC + 2) * D], FR)
        wo_sb = const.tile([128, HPC, INNER], FR)
        cos_sb = const.tile([D, T], FR)
        sin_sb = const.tile([D, T], FR)
        tri_sb = const.tile([128, 128], FR)
        ones_col = const.tile([128, 128], FR)
        nc.gpsimd.memset(ones_col, 1.0)
        # per-batch persistent activations (single-buffered, reused)
        qkt = data.tile([128, HPC + 1, T], FR)   # 4 q heads + k, [d, t]
        v_sb = data.tile([128, NKB, D], FR)      # [t(128-block), kb, d]
        ot = data.tile([128, HPC, T], FR)        # attention out, [d, h, t]

        # weight DMAs for ic-group g are emitted by the proj quanta below so
        # the first matmuls can start after ~2 DMAs instead of all of them.
        wdma_done = [False] * 8
        kvdma_done = [False] * 4
        cdma_done = [False]

        def emit_wdma(hg):
            if wdma_done[hg]:
                return
            wdma_done[hg] = True
            nsub = 2 if hg == 0 else 1
            for s in range(nsub):
                i0 = hg * 4 + s * 4 // nsub
                i1 = hg * 4 + (s + 1) * 4 // nsub
                nc.sync.dma_start(
                    out=w_sb[:, i0:i1, : HPC * D],
                    in_=wq[i0 * 128:i1 * 128, :].rearrange(
                        "(i p) c -> p i c", p=128))

        def emit_kvdma(g):
            if kvdma_done[g]:
                return
            kvdma_done[g] = True
            i0, i1 = g * 8, (g + 1) * 8
            nc.sync.dma_start(
                out=w_sb[:, i0:i1, HPC * D:],
                in_=wkv[i0 * 128:i1 * 128, :].rearrange(
                    "(i p) c -> p i c", p=128))

        def emit_cdma():
            if cdma_done[0]:
                return
            cdma_done[0] = True
            nc.sync.dma_start(out=cos_sb, in_=cosT)
            nc.sync.dma_start(out=sin_sb, in_=sinTs)
            nc.sync.dma_start(out=tri_sb, in_=triT)

        def wo_dma(h):
            def fn():
                nc.sync.dma_start(out=wo_sb[:, h, :],
                                  in_=wo[h * 128:(h + 1) * 128, :])
            return fn

        # ---------------- emission machinery ----------------
        items = deque()          # (tag, pe_ns, act_ns, fn)
        tagcnt = {}
        debt = [0.0]

        def push(tag, pe_ns, act_ns, fn):
            items.append((tag, pe_ns, act_ns, fn))
            tagcnt[tag] = tagcnt.get(tag, 0) + 1

        def emit_head(woven=False):
            tag, pe_ns, act_ns, fn = items.popleft()
            tagcnt[tag] -= 1
            fn()
            if woven:
                debt[0] = debt[0] - pe_ns + act_ns

        def weave(ns):
            debt[0] += ns
            while items and debt[0] >= items[0][1]:
                emit_head(woven=True)

        def flush_tag(tag):
            while tagcnt.get(tag, 0) > 0:
                emit_head()
            debt[0] = 0.0

        def flush_all():
            while items:
                emit_head()
            debt[0] = 0.0

        # shared x-tile cache: (b, tg, hg) -> tile; allocated+DMA'd on
        # first use; wave-2 quanta prefetch the next tg's chunk.
        xcache = {}

        def get_x(b, tg, hg):
            key = (b, tg, hg)
            if key not in xcache:
                xt = xp.tile([128, 4, QG], FR, tag="x", name="xt")
                t0x = b * T + tg * QG
                nsub = 2 if (b, tg, hg) == (0, 0, 0) else 1
                for s in range(nsub):
                    i0 = hg * 4 + s * 4 // nsub
                    i1 = hg * 4 + (s + 1) * 4 // nsub
                    nc.sync.dma_start(
                        out=xt[:, i0 - hg * 4:i1 - hg * 4, :],
                        in_=xT[i0 * 128:i1 * 128,
                               t0x:t0x + QG].rearrange(
                            "(i p) c -> p i c", p=128))
                xcache[key] = xt
            return xcache[key]

        # ---------------- projections ----------------
        # waves of 2 outputs over 32 ic chunks; outputs: (q0,q1),(q2,q3),(k,v)
        def proj_quanta(b, tg):
            """Append proj quanta for (b, tg) to `items` with tag."""
            tag = (b, "P", tg)
            t0 = b * T + tg * QG
            st = {"p": [None, None], "p5": None, "vt": None}
            nxt = (b, tg + 1) if tg + 1 < NQG else (b + 1, 0)

            def mmq(w, hg, first):
                def fn():
                    emit_wdma(hg)
                    if w == 1:
                        emit_kvdma(hg // 2)
                    if w == 2 and nxt[0] < B:
                        get_x(nxt[0], nxt[1], hg)  # prefetch next tg
                    xt = get_x(b, tg, hg)
                    if first:
                        st["p"][0] = pps.tile([128, QG], F32, tag="p0",
                                              name="p0")
                        st["p"][1] = pps.tile([128, QG], F32, tag="p1",
                                              name="p1")
                    for j in range(2):
                        o = 2 * w + j
                        for ici in range(4):
                            ic = hg * 4 + ici
                            nc.tensor.matmul(
                                st["p"][j],
                                w_sb[:, ic, o * D:(o + 1) * D],
                                xt[:, ici, :],
                                start=(ic == 0), stop=(ic == NIC - 1))
                return fn

            def copyq(w):
                def fn():
                    emit_cdma()
                    if st["p5"] is None:
                        st["p5"] = wk_.tile([128, HPC + 1, QG], FR, tag="p5",
                                            name="p5")
                    for j in range(2):
                        o = 2 * w + j
                        if o < HPC + 1:
                            nc.vector.tensor_copy(st["p5"][:, o, :], st["p"][j])
                        else:  # v
                            vt = wk_.tile([128, QG], FR, tag="vt", bufs=1,
                                          name="vt")
                            nc.vector.tensor_copy(vt, st["p"][j])
                            st["vt"] = vt
                return fn

            def ropeq():
                def fn():
                    p5 = st["p5"]
                    rot = wk_.tile([128, HPC + 1, QG], FR, tag="rot",
                                   bufs=1, name="rot")
                    nc.sync.dma_start(out=rot[0:64], in_=p5[64:128])
                    nc.sync.dma_start(out=rot[64:128], in_=p5[0:64])
                    cs = cos_sb[:, None, tg * QG:(tg + 1) * QG].broadcast_to(
                        (128, HPC + 1, QG))
                    ss = sin_sb[:, None, tg * QG:(tg + 1) * QG].broadcast_to(
                        (128, HPC + 1, QG))
                    a5 = wk_.tile([128, HPC + 1, QG], FR, tag="a5",
                                  bufs=1, name="a5")
                    nc.vector.tensor_mul(a5, p5, cs)
                    nc.vector.tensor_mul(rot, rot, ss)
                    nc.vector.tensor_add(
                        qkt[:, :, tg * QG:(tg + 1) * QG], a5, rot)
                    # V: XBAR transpose [d, t] -> [t(128), 4, d]
                    nc.sync.dma_start_transpose(
                        v_sb[:, tg * 4:tg * 4 + 4, :], st["vt"])
                return fn

            for w in range(3):
                for hg in range(8):
                    push(tag, 1707 + 100, 0, mmq(w, hg, hg == 0))
                push(tag, 0, 1300, copyq(w))
            push(tag, 0, 0, ropeq())

        # ---------------- WO units ----------------
        def wo_quanta(b, qg):
            tag = (b, "W", qg)
            t0 = b * T
            st = {"oo": None}

            def unit(tb, cg):
                def fn():
                    op = wps.tile([128, 512], F32, tag="w", name="op")
                    for h in range(HPC):
                        nc.tensor.matmul(
                            op,
                            ot[:, h, tb * 128:(tb + 1) * 128],
                            wo_sb[:, h, cg * 512:(cg + 1) * 512],
                            start=(h == 0), stop=(h == HPC - 1))
                    if cg % 2 == 0:
                        st["oo"] = wk_.tile([128, 2, 512], FR, tag="oo",
                                            bufs=3, name="oo")
                    nc.vector.tensor_copy(st["oo"][:, cg % 2, :], op)
                    if cg % 2 == 1:
                        q = cg // 2
                        nc.sync.dma_start(
                            out=out[t0 + tb * 128:t0 + (tb + 1) * 128,
                                    q * 1024:(q + 1) * 1024],
                            in_=st["oo"])
                return fn

            for tb in range(qg * 4, qg * 4 + 4):
                for cg in range(NCG):
                    push(tag, 852 + 120, 580, unit(tb, cg))

        # ---------------- attention ----------------
        def attn(b, h, qg):
            qs = qkt[:, h, qg * QG:(qg + 1) * QG]
            kmax = 4 * qg + 4
            o_ps = ops.tile([D, QG], F32, tag="o", name="o_ps")
            ea = wk_.tile([128, QG], FR, tag="ea", name="ea")
            pend = []

            def emit_o(kb2, q02, e2):
                nc.tensor.matmul(
                    o_ps[:, q02:], v_sb[:, kb2, :], e2[:, q02:],
                    start=(kb2 == 0), stop=(kb2 == kmax - 1))

            for kb in range(kmax):
                q0 = max(0, 128 * (kb - 4 * qg))
                s_ps = sps.tile([128, QG], F32, tag="s", name="s_ps")
                nc.tensor.matmul(
                    s_ps[:, q0:],
                    qkt[:, HPC, kb * 128:(kb + 1) * 128],
                    qs[:, q0:], start=True, stop=True)
                e = wk_.tile([128, QG], FR, tag="e", bufs=5, name="e")
                nc.scalar.activation(e[:, q0:], s_ps[:, q0:], EXP,
                                     bias=biasm4)
                if kb >= 4 * qg:
                    nc.vector.tensor_mul(e[:, q0:q0 + 128],
                                         e[:, q0:q0 + 128], tri_sb)
                if kb == 0:
                    nc.vector.tensor_copy(ea, e)
                else:
                    nc.vector.tensor_add(ea[:, q0:], ea[:, q0:], e[:, q0:])
                pend.append((kb, q0, e))
                # PV lags the score stream by one block so the exp latency
                # never stalls the PE
                if len(pend) > 3:
                    emit_o(*pend.pop(0))
                weave(550)
            while pend:
                emit_o(*pend.pop(0))
            if items:
                emit_head()
            z_ps = ops.tile([128, QG], F32, tag="o", name="z_ps")
            nc.tensor.matmul(z_ps, ones_col, ea, start=True, stop=True)
            r = wk_.tile([128, QG], F32, tag="r", bufs=1, name="r")
            nc.vector.reciprocal(r, z_ps)
            nc.vector.tensor_mul(ot[:, h, qg * QG:(qg + 1) * QG], o_ps, r)
            weave(550)

        # ---------------- program ----------------
        proj_quanta(0, 0)
        flush_tag((0, "P", 0))
        proj_quanta(0, 1)
        flush_tag((0, "P", 1))
        for h in range(HPC):
            push("wodma", 1707, 0, wo_dma(h))
        proj_quanta(0, 2)
        proj_quanta(0, 3)
        steps = [(b, qg) for b in range(B) for qg in range(NQG)]
        for i, (b, qg) in enumerate(steps):
            flush_tag((b, "P", qg))  # no-op when already prefetch-flushed
            for h in range(HPC):
                attn(b, h, qg)
            wo_quanta(b, qg)
            if (b, qg) == (0, 3):
                for tg in range(NQG):
                    proj_quanta(1, tg)
            # flush the NEXT step's projection now: its RoPE chain
            # (ACT copies -> rot DMA -> DVE) overlaps this step's
            # still-executing attention instead of stalling the PE.
            if i + 1 < len(steps):
                nb, nqg = steps[i + 1]
                flush_tag((nb, "P", nqg))
        flush_all()
    _split_waits(nc, mybir)
    return nc


def _build_generic():
    import concourse.bass as bass
    import concourse.mybir as mybir
    import concourse.tile as tile
    from concourse.masks import make_identity
    from contextlib import ExitStack

    F32 = mybir.dt.float32
    FR = mybir.dt.float16  # matmul-operand dtype (10-bit mantissa)
    EXP = mybir.ActivationFunctionType.Exp

    nc = bass.Bass(trn_type="TRN2", target_bir_lowering=False, debug=False)
    xT = nc.dram_tensor("xT", [INNER, BT], FR, kind="ExternalInput").ap()
    wq = nc.dram_tensor("wq", [INNER, HPC * D], FR, kind="ExternalInput").ap()
    wkv = nc.dram_tensor("wkv", [INNER, 2 * D], FR, kind="ExternalInput").ap()
    wo = nc.dram_tensor("wo", [HPC * D, INNER], FR, kind="ExternalInput").ap()
    cosT = nc.dram_tensor("cosT", [D, T], F32, kind="ExternalInput").ap()
    sinTs = nc.dram_tensor("sinTs", [D, T], F32, kind="ExternalInput").ap()
    mwTf = nc.dram_tensor("mwTf", [T, T], F32, kind="ExternalInput").ap()
    mbTf = nc.dram_tensor("mbTf", [T, T], F32, kind="ExternalInput").ap()
    out = nc.dram_tensor("out", [BT, INNER], F32, kind="ExternalOutput").ap()

    with tile.TileContext(nc) as tc, ExitStack() as ctx:
        const = ctx.enter_context(tc.tile_pool(name="const", bufs=1))
        cos_sb = const.tile([D, T], F32)
        sin_sb = const.tile([D, T], F32)
        nc.sync.dma_start(out=cos_sb, in_=cosT)
        nc.sync.dma_start(out=sin_sb, in_=sinTs)
        ones_col = const.tile([128, 128], FR)
        nc.gpsimd.memset(ones_col, 1.0)
        ident = const.tile([128, 128], F32)
        make_identity(nc, ident)
        biasm4 = const.tile([128, 1], F32)
        nc.gpsimd.memset(biasm4, -4.0)

        for b in range(B):
            t0 = b * T
            with tc.tile_pool(name=f"bp{b}", bufs=1) as bp:
                qt_sb = [bp.tile([D, T], FR, tag=f"qt{h}", name=f"qt{h}")
                         for h in range(HPC)]
                kt_sb = bp.tile([D, T], FR, tag="kt")
                v_sb = bp.tile([128, NKB, D], FR, tag="v")
                with tc.tile_pool(name="wproj", bufs=1) as wpool, \
                     tc.tile_pool(name="xin", bufs=8) as xpool, \
                     tc.tile_pool(name="peps", bufs=3) as epool, \
                     tc.tile_pool(name="pps", bufs=1, space="PSUM") as pps, \
                     tc.tile_pool(name="tps", bufs=2, space="PSUM") as tps:
                    w_sb = wpool.tile([128, NIC, (HPC + 2) * D], FR)
                    for ic in range(NIC):
                        nc.sync.dma_start(
                            out=w_sb[:, ic, : HPC * D],
                            in_=wq[ic * 128:(ic + 1) * 128, :])
                        nc.sync.dma_start(
                            out=w_sb[:, ic, HPC * D:],
                            in_=wkv[ic * 128:(ic + 1) * 128, :])
                    for tg in range(NQG):
                        tc0 = t0 + tg * QG
                        prj = [pps.tile([128, QG], F32, tag=f"prj{i}",
                                        name=f"prj{i}")
                               for i in range(HPC + 2)]
                        for ic in range(NIC):
                            x_sb = xpool.tile([128, QG], FR)
                            nc.sync.dma_start(
                                out=x_sb,
                                in_=xT[ic * 128:(ic + 1) * 128, tc0:tc0 + QG])
                            for i in range(HPC + 2):
                                nc.tensor.matmul(
                                    prj[i],
                                    w_sb[:, ic, i * D:(i + 1) * D],
                                    x_sb,
                                    start=(ic == 0), stop=(ic == NIC - 1))
                        cs = cos_sb[:, tg * QG:(tg + 1) * QG]
                        ss = sin_sb[:, tg * QG:(tg + 1) * QG]
                        for i in range(HPC + 1):  # 4 q heads + k
                            ps = prj[i]
                            p_sb = epool.tile([128, QG], F32, tag="psb")
                            nc.scalar.copy(p_sb, ps)
                            rot = epool.tile([128, QG], F32, tag="rot")
                            nc.sync.dma_start(out=rot[0:64, :],
                                              in_=p_sb[64:128, :])
                            nc.sync.dma_start(out=rot[64:128, :],
                                              in_=p_sb[0:64, :])
                            a_t = epool.tile([128, QG], F32, tag="ropea")
                            nc.vector.tensor_mul(a_t, p_sb, cs)
                            b_t = epool.tile([128, QG], F32, tag="ropeb")
                            nc.vector.tensor_mul(b_t, rot, ss)
                            dst = qt_sb[i] if i < HPC else kt_sb
                            nc.vector.tensor_add(
                                dst[:, tg * QG:(tg + 1) * QG], a_t, b_t)
                        vtmp = epool.tile([128, QG], F32, tag="vtmp")
                        nc.scalar.copy(vtmp, prj[HPC + 1])
                        for j in range(QG // 128):
                            vt_ps = tps.tile([128, 128], F32, tag="vt")
                            nc.tensor.transpose(
                                vt_ps, vtmp[:, j * 128:(j + 1) * 128], ident)
                            nc.vector.tensor_copy(v_sb[:, tg * 4 + j, :], vt_ps)

                ot_sb = [bp.tile([D, T], FR, tag=f"ot{h}", name=f"ot{h}")
                         for h in range(HPC)]
                with tc.tile_pool(name="amask", bufs=1) as mpool, \
                     tc.tile_pool(name="exps", bufs=24) as spool, \
                     tc.tile_pool(name="asml", bufs=4) as apool, \
                     tc.tile_pool(name="sps", bufs=5, space="PSUM") as sps, \
                     tc.tile_pool(name="ops", bufs=2, space="PSUM") as ops, \
                     tc.tile_pool(name="zps", bufs=1, space="PSUM") as zps:
                    for h in range(HPC):
                        for qg in range(NQG):
                            qs = qt_sb[h][:, qg * QG:(qg + 1) * QG]
                            kmax = NKB
                            o_ps = ops.tile([D, QG], F32, tag="o")
                            z_ps = zps.tile([128, QG], F32, tag="z")
                            for kb in range(kmax):
                                s_ps = sps.tile([128, QG], F32, tag="s")
                                nc.tensor.matmul(
                                    s_ps,
                                    kt_sb[:, kb * 128:(kb + 1) * 128],
                                    qs, start=True, stop=True)
                                mw_t = apool.tile([128, QG], F32, tag="mw")
                                nc.sync.dma_start(
                                    out=mw_t,
                                    in_=mwTf[kb * 128:(kb + 1) * 128,
                                             qg * QG:(qg + 1) * QG])
                                mb_t = apool.tile([128, QG], F32, tag="mb")
                                nc.sync.dma_start(
                                    out=mb_t,
                                    in_=mbTf[kb * 128:(kb + 1) * 128,
                                             qg * QG:(qg + 1) * QG])
                                nc.vector.tensor_mul(s_ps, s_ps, mw_t)
                                nc.vector.tensor_add(s_ps, s_ps, mb_t)
                                e_sb = spool.tile([128, QG], FR, tag="e")
                                nc.scalar.activation(e_sb, s_ps, EXP,
                                                     bias=biasm4)
                                nc.tensor.matmul(
                                    z_ps, ones_col, e_sb,
                                    start=(kb == 0), stop=(kb == kmax - 1))
                                nc.tensor.matmul(
                                    o_ps, v_sb[:, kb, :], e_sb,
                                    start=(kb == 0), stop=(kb == kmax - 1))
                            r_sb = apool.tile([128, QG], F32, tag="r")
                            nc.vector.reciprocal(r_sb, z_ps)
                            nc.vector.tensor_mul(
                                ot_sb[h][:, qg * QG:(qg + 1) * QG],
                                o_ps, r_sb)

                with tc.tile_pool(name="wom", bufs=2) as wopool, \
                     tc.tile_pool(name="wos", bufs=6) as wosb, \
                     tc.tile_pool(name="wops", bufs=4, space="PSUM") as wps:
                    for cg in range(NCG):
                        wo_sb = wopool.tile([128, HPC, 512], FR, tag="wo")
                        for h in range(HPC):
                            nc.sync.dma_start(
                                out=wo_sb[:, h, :],
                                in_=wo[h * 128:(h + 1) * 128,
                                       cg * 512:(cg + 1) * 512])
                        for tb in range(NTB):
                            op = wps.tile([128, 512], F32, tag="op")
                            for h in range(HPC):
                                nc.tensor.matmul(
                                    op,
                                    ot_sb[h][:, tb * 128:(tb + 1) * 128],
                                    wo_sb[:, h, :],
                                    start=(h == 0), stop=(h == HPC - 1))
                            o_sb = wosb.tile([128, 512], F32, tag="osb")
                            nc.any.tensor_copy(o_sb, op)
                            nc.sync.dma_start(
                                out=out[t0 + tb * 128:t0 + (tb + 1) * 128,
                                        cg * 512:(cg + 1) * 512],
                                in_=o_sb)
    _split_waits(nc, mybir)
    return nc


def _build(variant):
    if variant == "causal":
        return _build_causal()
    return _build_generic()


def _get(variant):
    if variant not in _built:
        _built[variant] = _build(variant)
    return _built[variant]


def _canonical_causal(mask_w, mask_b):
    tri = np.tril(np.ones((T, T), dtype=np.float32))
    if not np.array_equal(mask_w, tri):
        return False
    off = mask_b[tri == 0]
    if off.size and not (np.all(off <= -20000.0) and np.ptp(off) == 0):
        return False
    return bool(np.all(mask_b[tri == 1] == 0.0))


def _run(stm, wq, wk, wv, wo, cos, sin, mask_w, mask_b, trace=False):
    from concourse.bass_utils import run_bass_kernel_spmd

    BF = np.float16
    x = np.ascontiguousarray(np.asarray(stm).reshape(BT, INNER))
    xT = np.ascontiguousarray(x.T).astype(BF)
    wq_s = (np.asarray(wq) * ATTN_SCALE).astype(BF)
    wk = np.asarray(wk); wv = np.asarray(wv); wo = np.asarray(wo)
    cosT = np.ascontiguousarray(np.asarray(cos).T)
    sinT = np.ascontiguousarray(np.asarray(sin).T)
    sinTs = sinT.copy()
    sinTs[: D // 2] *= -1.0
    mask_w = np.asarray(mask_w); mask_b = np.asarray(mask_b)
    causal = _canonical_causal(mask_w, mask_b)
    variant = "causal" if causal else "generic"
    nc = _get(variant)

    in_maps = []
    for c in range(NCORES):
        m = {
            "xT": xT,
            "wq": np.ascontiguousarray(wq_s[:, c * HPC * D:(c + 1) * HPC * D]),
            "wkv": np.ascontiguousarray(
                np.concatenate([wk[:, c * D:(c + 1) * D],
                                wv[:, c * D:(c + 1) * D]], axis=1)).astype(BF),
            "wo": np.ascontiguousarray(
                wo[c * HPC * D:(c + 1) * HPC * D, :]).astype(BF),
        }
        if causal:
            m["cosT"] = cosT.astype(BF)
            m["sinTs"] = sinTs.astype(BF)
            # tri[k, q] = 1 where k <= q (within the diagonal 128 block)
            m["triT"] = np.triu(np.ones((128, 128), dtype=BF))
        else:
            m["cosT"] = cosT
            m["sinTs"] = sinTs
            m["mwTf"] = np.ascontiguousarray(mask_w.T)
            m["mbTf"] = np.ascontiguousarray(mask_b.T)
        in_maps.append(m)

    res = run_bass_kernel_spmd(nc, in_maps, core_ids=list(range(NCORES)),
                               trace=trace)
    acc = res.results[0]["out"].astype(np.float64)
    for c in range(1, NCORES):
        acc += res.results[c]["out"]
    full = acc.astype(np.float32).reshape(B, T, H, D)
    return full, res


def kernel(stm, wq, wk, wv, wo, cos, sin, mask_w, mask_b):
    out, _ = _run(stm, wq, wk, wv, wo, cos, sin, mask_w, mask_b, trace=False)
    return out
